# revision 14
# baseline (speedup 1.0000x reference)
"""Trainium2 Bass kernel for one CLIP transformer layer (pre-LN causal
attention + GELU FFN), data-parallel over batch across 8 NeuronCores.

Strategy (per core, one batch element, everything feature-major ["transposed"]
[d, s] so matmul contractions always run over the partition dim):

  host:  transpose x -> xT, pre-transpose / LN-fold all weights, fold biases
  LN1:   stats via ones-matmul column sums (+ x^2 pass), K=1 matmul broadcast,
         apply on DVE -> h1T [D, S]
  QKV:   qT/kT per head-pair via W^T-stationary matmuls; V in natural [s, d]
         layout (with an appended ones column for softmax row sums)
  attn:  per head-pair, scores^T = K-tile @ Q^T row-packed (two K=64 matmuls
         sharing the PE array), additive causal band mask on the diagonal
         tiles only, exp on ScalarE, attn^T @ [V|1] accumulated in PSUM with
         ragged (causality-trimmed) column ranges; softmax normalization via
         DVE reciprocal of the rowsum row + K=1 matmul partition-broadcast
  proj:  out-proj + residual (fused scalar_tensor_tensor), LN2, FFN with
         gelu-tanh fused into the FFN1 PSUM evacuation, FFN2 + residual
  all matmuls in float32r (full-rate on TRN2; measured ~1.5e-4 absmax err)
"""
import math
from contextlib import ExitStack

import numpy as np

import concourse.bass as bass
import concourse.mybir as mybir
import concourse.tile as tile
from concourse import bacc
from concourse.bass_utils import run_bass_kernel_spmd

B, S, D, H, FF = 8, 1024, 1024, 16, 4096
DH = D // H
EPS = 1e-5
P = 128
QC = 512                 # q-chunk width == one fp32 PSUM bank
NEG = -1e10              # additive causal mask value

f32 = mybir.dt.float32
f32r = mybir.dt.float32r
bf16 = mybir.dt.bfloat16

# which matmul families run in bf16 (1 cyc/row) vs float32r (2 cyc/row)
BF16_FFN = True     # h2T, w1, w2, a
BF16_ATTN = True    # qt/kt, scores, V, exp(attn), AV
BF16_PROJ = True    # h1T, wqk, wv, wo, oT, out-projection
ALU = mybir.AluOpType
ACTF = mybir.ActivationFunctionType

TRACE = False            # set by test.py for profiled runs
LAST_RESULTS = None      # BassKernelResults of the most recent run


class _Pool:
    """A tile pool with an explicit close() so SBUF is reclaimed mid-kernel
    (TileContext queue allocation mode reuses released ranges FIFO)."""

    def __init__(self, tc, **kw):
        self._cm = tc.tile_pool(**kw)
        self.pool = self._cm.__enter__()

    def tile(self, *a, **kw):
        if "name" not in kw:
            kw["name"] = kw.get("tag") or "t"
        return self.pool.tile(*a, **kw)

    def close(self):
        self._cm.__exit__(None, None, None)


def _layernorm_t(nc, tc, x_t, h_t, dc, s, ones_p1, ones_1p):
    """LayerNorm over the partition (feature) axis of x_t [128, dc, s],
    writing h_t = (x - mu) * rstd in the same layout. gamma/beta are folded
    into the downstream weights on the host."""
    nq = s // QC
    d = dc * P
    with tc.tile_pool(name="ln_sb", bufs=2) as lnp, \
         tc.tile_pool(name="ln_sb1", bufs=1) as lnp1, \
         tc.tile_pool(name="ln_ps", bufs=1, space="PSUM") as lps:
        ps_sx = lps.tile([1, s], f32, tag="sx")
        ps_sxx = lps.tile([1, s], f32, tag="sxx")
        for c in range(dc):
            xsq = lnp.tile([P, s], f32r, tag="xsq")
            nc.scalar.activation(xsq, x_t[:, c, :], ACTF.Square)
            for q in range(nq):
                sl = slice(q * QC, (q + 1) * QC)
                nc.tensor.matmul(ps_sx[:, sl], ones_p1, x_t[:, c, sl],
                                 start=(c == 0), stop=(c == dc - 1))
                nc.tensor.matmul(ps_sxx[:, sl], ones_p1, xsq[:, sl],
                                 start=(c == 0), stop=(c == dc - 1))
        sx = lnp1.tile([1, s], f32r, tag="ssx")
        sxx = lnp1.tile([1, s], f32r, tag="ssxx")
        nc.scalar.copy(sx, ps_sx)
        nc.scalar.copy(sxx, ps_sxx)

        ps_bx = lps.tile([P, s], f32, tag="bcx")
        ps_bxx = lps.tile([P, s], f32, tag="bcxx")
        for q in range(nq):
            sl = slice(q * QC, (q + 1) * QC)
            nc.tensor.matmul(ps_bx[:, sl], ones_1p, sx[:, sl],
                             start=True, stop=True)
            nc.tensor.matmul(ps_bxx[:, sl], ones_1p, sxx[:, sl],
                             start=True, stop=True)

        # rstd = d / sqrt(d*Sxx - Sx^2 + d^2 eps);  h = x*(d*rr) - Sx*rr
        a2 = lnp1.tile([P, s], f32, tag="a2")
        nc.scalar.activation(a2, ps_bx, ACTF.Square)
        m = lnp1.tile([P, s], f32, tag="m")
        nc.vector.tensor_scalar_mul(m, ps_bxx, float(d))
        nc.vector.tensor_sub(m, m, a2)
        sd = lnp1.tile([P, s], f32, tag="sd")
        eps_sb = lnp1.tile([P, 1], f32, tag="eps")
        nc.vector.memset(eps_sb, float(d) * d * EPS)
        nc.scalar.activation(sd, m, ACTF.Sqrt, bias=eps_sb)
        rr = lnp1.tile([P, s], f32, tag="rr")
        nc.vector.reciprocal(rr, sd)
        rs = lnp1.tile([P, s], f32, tag="rs")
        nc.vector.tensor_scalar_mul(rs, rr, float(d))
        m2 = lnp1.tile([P, s], f32, tag="m2")
        nc.vector.tensor_mul(m2, ps_bx, rr)
        for c in range(dc):
            eng = nc.vector if c % 2 == 0 else nc.gpsimd
            tmp = lnp.tile([P, s], f32, tag="app" + str(c % 2), name="app")
            eng.tensor_mul(tmp, x_t[:, c, :], rs)
            eng.tensor_sub(h_t[:, c, :], tmp, m2)


def build_nc(s=S):
    """Build the per-core Bass program (SPMD; identical on all 8 cores)."""
    dc = D // P              # feature chunks
    nq = s // QC             # q chunks
    kts = s // P             # k tiles
    nhp = H // 2             # head pairs
    nft = FF // P            # FFN hidden tiles
    kpq = QC // P            # k-tiles per q-chunk

    nc = bacc.Bacc()
    xT = nc.declare_dram_parameter("xT", [D, s], f32r, isOutput=False)
    dt_proj = bf16 if BF16_PROJ else f32r
    dt_ffn = bf16 if BF16_FFN else f32r
    dt_attn = bf16 if BF16_ATTN else f32r
    wqkT = nc.declare_dram_parameter("wqkT", [D, 2 * D], dt_proj,
                                     isOutput=False)
    wvT = nc.declare_dram_parameter("wvT", [D, D], dt_proj, isOutput=False)
    woT = nc.declare_dram_parameter("woT", [D, D], dt_proj, isOutput=False)
    w1T = nc.declare_dram_parameter("w1T", [D, FF], dt_ffn, isOutput=False)
    w2T = nc.declare_dram_parameter("w2T", [FF, D], dt_ffn, isOutput=False)
    bqk = nc.declare_dram_parameter("bqk", [P, 2 * dc], f32, isOutput=False)
    bo = nc.declare_dram_parameter("bo", [P, dc], f32, isOutput=False)
    b1 = nc.declare_dram_parameter("b1", [P, nft], f32, isOutput=False)
    b2 = nc.declare_dram_parameter("b2", [P, dc], f32, isOutput=False)
    mk = nc.declare_dram_parameter("mk", [P, P], f32, isOutput=False)
    onesd = nc.declare_dram_parameter("onesd", [P, P], f32r, isOutput=False)
    onesb = nc.declare_dram_parameter("onesb", [P, P], dt_attn, isOutput=False)
    outT = nc.declare_dram_parameter("outT", [D, s], f32, isOutput=True)

    def chunked(t):
        return t.rearrange("(c p) n -> p c n", p=P)

    with tile.TileContext(nc, pool_alloc_mode="queue") as tc:
        with tc.tile_pool(name="glob", bufs=1) as g:
            ones_p1 = g.tile([P, 1], f32r)
            nc.sync.dma_start(out=ones_p1, in_=onesd[:, 0:1])
            ones_164 = g.tile([1, DH], f32r)
            nc.sync.dma_start(out=ones_164, in_=onesd[0:1, 0:DH])
            ones_1p = g.tile([1, P], f32r)
            nc.sync.dma_start(out=ones_1p, in_=onesd[0:1, :])
            mask_sb = g.tile([P, P], f32)
            nc.sync.dma_start(out=mask_sb, in_=mk[:, :])
            bqk_sb = g.tile([P, 2 * dc], f32)
            nc.sync.dma_start(out=bqk_sb, in_=bqk[:, :])
            bo_sb = g.tile([P, dc], f32)
            nc.sync.dma_start(out=bo_sb, in_=bo[:, :])
            b1_sb = g.tile([P, nft], f32)
            nc.sync.dma_start(out=b1_sb, in_=b1[:, :])
            b2_sb = g.tile([P, dc], f32)
            nc.sync.dma_start(out=b2_sb, in_=b2[:, :])

            # pools opened in reverse close order (pool events are LIFO)
            xap = _Pool(tc, name="xattn", bufs=1)
            xattnT = xap.tile([P, dc, s], f32r, tag="xattnT")
            otp = _Pool(tc, name="ot", bufs=1)
            oT = otp.tile([P, nhp, s], dt_proj, tag="oT")

            # ---------------- LN1 ----------------
            h1p = _Pool(tc, name="h1", bufs=1)
            h1T = h1p.tile([P, dc, s], dt_proj, tag="h1T")
            xin = _Pool(tc, name="xin", bufs=1)
            xt = xin.tile([P, dc, s], f32r, tag="xt")
            nc.sync.dma_start(out=xt, in_=chunked(xT))
            _layernorm_t(nc, tc, xt, h1T, dc, s, ones_p1, ones_1p)
            xin.close()

            # ------------- V = h @ WvT (natural layout, + ones col) -------
            vp = _Pool(tc, name="v", bufs=1)
            v_sb = vp.tile([P, kts, H, DH + 1], dt_attn, tag="v_sb")
            with tc.tile_pool(name="wv", bufs=1) as wvp, \
                 tc.tile_pool(name="vps", bufs=3, space="PSUM") as vps:
                wv_sb = wvp.tile([P, dc, D], dt_proj)
                nc.sync.dma_start(out=wv_sb, in_=chunked(wvT))
                hh = QC // DH  # heads per v-chunk
                for st in range(kts):
                    for vc in range(D // QC):
                        pv = vps.tile([P, QC], f32, tag="pv")
                        for c in range(dc):
                            nc.tensor.matmul(
                                pv, h1T[:, c, st * P:(st + 1) * P],
                                wv_sb[:, c, vc * QC:(vc + 1) * QC],
                                start=(c == 0), stop=(c == dc - 1))
                        nc.scalar.copy(
                            v_sb[:, st, vc * hh:(vc + 1) * hh, 0:DH],
                            pv.rearrange("p (h e) -> p h e", h=hh))
                nc.sync.dma_start(
                    out=v_sb[:, :, :, DH:DH + 1],
                    in_=onesb[:, 0:kts * H].rearrange(
                        "p (k h o) -> p k h o", k=kts, h=H))

            # ---------------- attention, per head pair ----------------
            with tc.tile_pool(name="wqk", bufs=3) as wqkp, \
                 tc.tile_pool(name="qk", bufs=2) as qkp, \
                 tc.tile_pool(name="at", bufs=4) as atp, \
                 tc.tile_pool(name="nrm", bufs=2) as nrmp, \
                 tc.tile_pool(name="qps", bufs=2, space="PSUM") as qps, \
                 tc.tile_pool(name="sps", bufs=3, space="PSUM") as sps, \
                 tc.tile_pool(name="ops", bufs=2, space="PSUM") as ops, \
                 tc.tile_pool(name="bps", bufs=1, space="PSUM") as bps:
                wqk_ch = chunked(wqkT)
                for hp in range(nhp):
                    qt = qkp.tile([P, s], dt_attn, tag="qt")
                    kt = qkp.tile([P, s], dt_attn, tag="kt")
                    for which, dst in ((0, qt), (1, kt)):
                        wt = wqkp.tile([P, dc, P], dt_proj, tag="w")
                        o0 = which * D + hp * P
                        nc.sync.dma_start(out=wt, in_=wqk_ch[:, :, o0:o0 + P])
                        for q in range(nq):
                            sl = slice(q * QC, (q + 1) * QC)
                            pq = qps.tile([P, QC], f32, tag="pq")
                            for c in range(dc):
                                nc.tensor.matmul(
                                    pq, wt[:, c, :], h1T[:, c, sl],
                                    start=(c == 0), stop=(c == dc - 1))
                            bcol = which * dc + hp
                            nc.scalar.activation(
                                dst[:, sl], pq, ACTF.Identity,
                                bias=bqk_sb[:, bcol:bcol + 1])
                    for q in range(nq):
                        sl = slice(q * QC, (q + 1) * QC)
                        po = [ops.tile([DH + 1, QC], f32, tag="po", name="po")
                              for _ in range(2)]
                        nkt = (q + 1) * kpq
                        for ki in range(nkt):
                            r = ki * P - q * QC
                            c0 = max(r, 0)
                            w = QC - c0
                            qsl = slice(q * QC + c0, (q + 1) * QC)
                            ats = []
                            for hb in range(2):
                                hsl = slice(hb * DH, (hb + 1) * DH)
                                ps = sps.tile([P, QC], f32, tag="ps")
                                nc.tensor.matmul(
                                    ps[:, 0:w], kt[hsl, ki * P:(ki + 1) * P],
                                    qt[hsl, qsl], start=True, stop=True)
                                if r >= 0:
                                    nc.vector.tensor_add(
                                        ps[:, 0:P], ps[:, 0:P], mask_sb)
                                at = atp.tile([P, QC], dt_attn, tag="at")
                                nc.scalar.activation(at[:, 0:w], ps[:, 0:w],
                                                     ACTF.Exp)
                                ats.append(at)
                            for hb in range(2):
                                nc.tensor.matmul(
                                    po[hb][:, c0:QC],
                                    v_sb[:, ki, 2 * hp + hb, :],
                                    ats[hb][:, 0:w],
                                    start=(ki == 0), stop=(ki == nkt - 1))
                        for hb in range(2):
                            rs = nrmp.tile([DH + 1, QC], f32, tag="rs")
                            nc.vector.reciprocal(rs[DH:DH + 1, :],
                                                 po[hb][DH:DH + 1, :])
                            r0 = nrmp.tile([1, QC], f32r, tag="r0")
                            nc.sync.dma_start(
                                out=r0, in_=rs[DH:DH + 1, :].bitcast(f32r))
                            pb = bps.tile([DH, QC], f32, tag="pb")
                            nc.tensor.matmul(pb, ones_164, r0[0:1, :],
                                             start=True, stop=True)
                            pbs = nrmp.tile([DH, QC], f32, tag="pbs")
                            nc.scalar.copy(pbs, pb)
                            if hb == 0:
                                nc.vector.tensor_mul(
                                    oT[0:DH, hp, sl], po[hb][0:DH, :], pbs)
                            else:
                                ob = nrmp.tile([DH, QC], dt_proj, tag="ob")
                                nc.vector.tensor_mul(ob, po[hb][0:DH, :], pbs)
                                nc.sync.dma_start(out=oT[DH:P, hp, sl],
                                                  in_=ob)
            vp.close()
            h1p.close()

            # ---------------- out-projection + residual ----------------
            with tc.tile_pool(name="wo", bufs=2) as wop, \
                 tc.tile_pool(name="xres", bufs=2) as xrp, \
                 tc.tile_pool(name="prs", bufs=3, space="PSUM") as prs:
                wo_ch = chunked(woT)
                xT_ch = chunked(xT)
                for ot in range(dc):
                    wt = wop.tile([P, dc, P], dt_proj, tag="wo")
                    nc.sync.dma_start(out=wt,
                                      in_=wo_ch[:, :, ot * P:(ot + 1) * P])
                    xr = xrp.tile([P, s], f32r, tag="xr")
                    nc.sync.dma_start(out=xr, in_=xT_ch[:, ot, :])
                    for q in range(nq):
                        sl = slice(q * QC, (q + 1) * QC)
                        pr = prs.tile([P, QC], f32, tag="pr")
                        for c in range(dc):
                            nc.tensor.matmul(pr, wt[:, c, :], oT[:, c, sl],
                                             start=(c == 0), stop=(c == dc - 1))
                        nc.vector.scalar_tensor_tensor(
                            xattnT[:, ot, sl], pr, bo_sb[:, ot:ot + 1],
                            xr[:, sl], op0=ALU.add, op1=ALU.add)
            otp.close()

            # ---------------- LN2 ----------------
            h2p = _Pool(tc, name="h2", bufs=1)
            h2T = h2p.tile([P, dc, s], dt_ffn, tag="h2T")
            _layernorm_t(nc, tc, xattnT, h2T, dc, s, ones_p1, ones_1p)

            # ---------------- FFN ----------------
            with tc.tile_pool(name="aff", bufs=nft + 4) as affp, \
                 tc.tile_pool(name="w1", bufs=3) as w1p, \
                 tc.tile_pool(name="w2", bufs=2) as w2p, \
                 tc.tile_pool(name="yout", bufs=3) as youtp, \
                 tc.tile_pool(name="aps", bufs=4, space="PSUM") as aps, \
                 tc.tile_pool(name="yps", bufs=3, space="PSUM") as yps:
                w1_ch = chunked(w1T)
                w2_ch = chunked(w2T)
                for q in range(nq):
                    sl = slice(q * QC, (q + 1) * QC)
                    a_tiles = []
                    for fc in range(nft):
                        wt = w1p.tile([P, dc, P], dt_ffn, tag="w1")
                        nc.sync.dma_start(
                            out=wt, in_=w1_ch[:, :, fc * P:(fc + 1) * P])
                        pa = aps.tile([P, QC], f32, tag="pa")
                        for c in range(dc):
                            nc.tensor.matmul(pa, wt[:, c, :], h2T[:, c, sl],
                                             start=(c == 0), stop=(c == dc - 1))
                        a = affp.tile([P, QC], dt_ffn, tag="a")
                        nc.scalar.activation(a, pa, ACTF.Gelu_apprx_tanh,
                                             bias=b1_sb[:, fc:fc + 1])
                        a_tiles.append(a)
                    nh = nft // 2
                    for do in range(dc):
                        py = yps.tile([P, QC], f32, tag="py")
                        for half in range(2):
                            wt = w2p.tile([P, nh, P], dt_ffn, tag="w2")
                            nc.sync.dma_start(
                                out=wt,
                                in_=w2_ch[:, half * nh:(half + 1) * nh,
                                          do * P:(do + 1) * P])
                            for fi in range(nh):
                                fc = half * nh + fi
                                nc.tensor.matmul(py, wt[:, fi, :], a_tiles[fc],
                                                 start=(fc == 0),
                                                 stop=(fc == nft - 1))
                        y = youtp.tile([P, QC], f32, tag="y")
                        nc.vector.scalar_tensor_tensor(
                            y, py, b2_sb[:, do:do + 1], xattnT[:, do, sl],
                            op0=ALU.add, op1=ALU.add)
                        nc.sync.dma_start(
                            out=outT[do * P:(do + 1) * P, sl], in_=y)
            h2p.close()
            xap.close()

    nc.compile()
    return nc


def prep_inputs(x, ln1_g, ln1_b, w_qkv, b_qkv, w_o, b_o, ln2_g, ln2_b,
                w1, b1, w2, b2, s=S):
    """Host-side preprocessing: LN gamma/beta folding, Q-scale folding,
    V-bias folding, transposes, per-tile bias layouts."""
    f = np.float32
    x = np.asarray(x, f)
    ln1_g, ln1_b = np.asarray(ln1_g, f), np.asarray(ln1_b, f)
    ln2_g, ln2_b = np.asarray(ln2_g, f), np.asarray(ln2_b, f)
    w_qkv, b_qkv = np.asarray(w_qkv, f), np.asarray(b_qkv, f)
    w_o, b_o = np.asarray(w_o, f), np.asarray(b_o, f)
    w1, b1 = np.asarray(w1, f), np.asarray(b1, f)
    w2, b2 = np.asarray(w2, f), np.asarray(b2, f)

    wqkv_e = w_qkv * ln1_g[None, :]
    bqkv_e = b_qkv + w_qkv @ ln1_b
    sc = f(1.0 / math.sqrt(DH))
    wq = wqkv_e[0:D] * sc
    bq = bqkv_e[0:D] * sc
    wk, bk = wqkv_e[D:2 * D], bqkv_e[D:2 * D]
    wv, bv = wqkv_e[2 * D:], bqkv_e[2 * D:]

    dcn = D // P
    import ml_dtypes
    npb = ml_dtypes.bfloat16
    tp = npb if BF16_PROJ else f
    tf_ = npb if BF16_FFN else f
    ta = npb if BF16_ATTN else f
    common = {
        "wqkT": np.ascontiguousarray(np.concatenate([wq, wk], 0).T).astype(tp),
        "wvT": np.ascontiguousarray(wv.T).astype(tp),
        "woT": np.ascontiguousarray(w_o.T).astype(tp),
        "w1T": np.ascontiguousarray((w1 * ln2_g[None, :]).T).astype(tf_),
        "w2T": np.ascontiguousarray(w2.T).astype(tf_),
        "bqk": np.ascontiguousarray(
            np.concatenate([bq, bk]).reshape(2 * dcn, P).T),
        "bo": np.ascontiguousarray((b_o + w_o @ bv).reshape(dcn, P).T),
        "b1": np.ascontiguousarray(
            (b1 + w1 @ ln2_b).reshape(FF // P, P).T),
        "b2": np.ascontiguousarray(b2.reshape(dcn, P).T),
        "mk": np.where(np.arange(P)[:, None] > np.arange(P)[None, :],
                       f(NEG), f(0.0)),
        "onesd": np.ones((P, P), f),
        "onesb": np.ones((P, P), ta),
    }
    in_maps = []
    for b in range(x.shape[0]):
        m = dict(common)
        m["xT"] = np.ascontiguousarray(x[b, :s].T)
        in_maps.append(m)
    return in_maps


_NC_CACHE = {}


def kernel(**inputs) -> np.ndarray:
    global LAST_RESULTS
    if S not in _NC_CACHE:
        _NC_CACHE[S] = build_nc(S)
    nc = _NC_CACHE[S]
    in_maps = prep_inputs(**inputs)
    res = run_bass_kernel_spmd(nc, in_maps, core_ids=list(range(B)),
                               trace=TRACE)
    LAST_RESULTS = res
    out = np.stack([res.results[b]["outT"].T for b in range(B)])
    return np.ascontiguousarray(out.astype(np.float32))
